# revision 3
# baseline (speedup 1.0000x reference)
"""AdaptiveFNO2d on 8 Trainium2 NeuronCores (axon/PJRT, data-parallel).

Sharding (per hint): batch B=128 split 8 ways (16/core); all params
replicated; FFTs local per core. The math is restructured so the whole
network is matmuls + GELU (no FFT primitive, which neuronx-cc cannot
compile):

  * rfft2/irfft2 are expressed as real DFT matmuls (64- and 126-point,
    twiddle matrices precomputed on host),
  * sigmoid(mode_weights) and the 1x1-conv mlp_w are folded into the
    per-mode spectral weights K_l = mw * (spec_w[l] + mlp_w[l].T)
    (exact: the 1x1 conv commutes with the FFT; mw here is constant so
    the rfft2->irfft2->rfft2 round trip is the identity on the
    weighted spectrum),
  * the encoder (CIN=3 -> WID=64) is folded into layer 0's per-mode
    weights (K=3 contraction); enc_b's DC contribution goes into the
    layer-0 bias,
  * spec_b + mlp_b fold into one per-layer bias before the exact-erf
    GELU; the decoder stays a [WID -> 1] channel matmul.

Device work per core/layer: 4 H-DFT einsums, 4 W-DFT einsums, the
per-mode channel mix (batched 64x64 matmuls over 4096 modes, re+im),
4+2 inverse-DFT einsums, bias+GELU.
"""

import sys

sys.path.insert(0, "/opt/trn_rl_repo")

import numpy as np

B, CIN, COUT, MM_, WID, L = 128, 3, 1, 64, 64, 4
H, W = 64, 126
WF = W // 2 + 1  # 64 rfft columns (kx=63 is the Nyquist bin, W even)
N_CORES = 8
BS = B // N_CORES

_jit_cache = {}


def _build_dft_mats():
    """Real/imag parts of the four DFT operators, float32.

    Fh [h, ky]   : forward DFT over H (rows)
    Fw [w, kx]   : forward rfft over W (cols), kx = 0..63
    Eh [ky, h]   : inverse DFT over H (includes 1/H)
    Cw [kx, w]   : inverse rfft over W (includes 1/W and the factor-2
                   Hermitian weights; kx=0 and kx=63=W/2 get weight 1)
    """
    h = np.arange(H)
    Fh = np.exp(-2j * np.pi * np.outer(h, h) / H)
    w = np.arange(W)
    kx = np.arange(WF)
    Fw = np.exp(-2j * np.pi * np.outer(w, kx) / W)
    Eh = np.exp(2j * np.pi * np.outer(h, h) / H) / H
    cwt = np.ones(WF)
    cwt[1 : WF - 1] = 2.0
    Cw = np.exp(2j * np.pi * np.outer(kx, w) / W) * (cwt[:, None] / W)
    f32 = np.float32
    return (
        f32(Fh.real), f32(Fh.imag), f32(Fw.real), f32(Fw.imag),
        f32(Eh.real), f32(Eh.imag), f32(Cw.real), f32(Cw.imag),
    )


def _fold_params(mode_weights, enc_w, enc_b, spec_w, spec_b, mlp_w, mlp_b):
    """Fold mw/mlp/enc into per-mode spectral weights (numpy, host)."""
    mw = 1.0 / (1.0 + np.exp(-np.float64(mode_weights)))  # [ky, kx]
    mwf = mw.astype(np.float32)[None, None]
    # K[l, i, o, ky, kx] = mw * (spec_w + mlp_w[l].T broadcast)
    K = np.empty((L, WID, WID, MM_, MM_), np.float32)
    for l in range(L):
        K[l] = (spec_w[l] + mlp_w[l].T[:, :, None, None]) * mwf[0]
    # layer 0: contract encoder in: K0[c, o, ky, kx]
    K0 = np.einsum("ic,ioyx->coyx", enc_w, K[0]).astype(np.float32)
    # enc_b DC contribution -> layer-0 bias (exact; enc_b is 0 here)
    b0_extra = np.einsum("i,io->o", enc_b, K[0][:, :, 0, 0]).astype(np.float32)
    biases = (spec_b + mlp_b).astype(np.float32)  # [L, WID]
    biases[0] += b0_extra
    return K0, K[1:], biases


def _make_fn():
    import jax
    import jax.numpy as jnp

    Fh_re, Fh_im, Fw_re, Fw_im, Eh_re, Eh_im, Cw_re, Cw_im = (
        jnp.asarray(m) for m in _build_dft_mats()
    )

    def rfft2(x):
        # x [b, c, h, w] real -> (re, im) [b, c, ky, kx]
        t_re = jnp.einsum("bchw,hy->bcyw", x, Fh_re)
        t_im = jnp.einsum("bchw,hy->bcyw", x, Fh_im)
        xf_re = jnp.einsum("bcyw,wx->bcyx", t_re, Fw_re) - jnp.einsum(
            "bcyw,wx->bcyx", t_im, Fw_im)
        xf_im = jnp.einsum("bcyw,wx->bcyx", t_re, Fw_im) + jnp.einsum(
            "bcyw,wx->bcyx", t_im, Fw_re)
        return xf_re, xf_im

    def irfft2(of_re, of_im):
        # (re, im) [b, o, ky, kx] -> x [b, o, h, w] real
        t_re = jnp.einsum("boyx,yh->bohx", of_re, Eh_re) - jnp.einsum(
            "boyx,yh->bohx", of_im, Eh_im)
        t_im = jnp.einsum("boyx,yh->bohx", of_re, Eh_im) + jnp.einsum(
            "boyx,yh->bohx", of_im, Eh_re)
        return jnp.einsum("bohx,xw->bohw", t_re, Cw_re) - jnp.einsum(
            "bohx,xw->bohw", t_im, Cw_im)

    def shard_fwd(x, K0, K, biases, dec_w, dec_b):
        # x [bs, CIN, H, W]
        for l in range(L):
            Kl = K0 if l == 0 else K[l - 1]
            xf_re, xf_im = rfft2(x)
            of_re = jnp.einsum("bixy,ioxy->boxy", xf_re, Kl)
            of_im = jnp.einsum("bixy,ioxy->boxy", xf_im, Kl)
            x = irfft2(of_re, of_im) + biases[l][None, :, None, None]
            x = jax.nn.gelu(x, approximate=False)
        out = jnp.einsum("bihw,oi->bohw", x, dec_w)
        return out + dec_b[None, :, None, None]

    if len(jax.devices()) >= N_CORES:
        return jax.pmap(shard_fwd, axis_name="cores",
                        in_axes=(0, None, None, None, None, None))
    # CPU fallback (single device): vmap over the shard axis
    return jax.jit(jax.vmap(shard_fwd, in_axes=(0, None, None, None, None, None)))


def kernel(**inputs):
    x = np.asarray(inputs["x"], np.float32)
    K0, K, biases = _fold_params(
        np.asarray(inputs["mode_weights"], np.float32),
        np.asarray(inputs["enc_w"], np.float32),
        np.asarray(inputs["enc_b"], np.float32),
        np.asarray(inputs["spec_w"], np.float32),
        np.asarray(inputs["spec_b"], np.float32),
        np.asarray(inputs["mlp_w"], np.float32),
        np.asarray(inputs["mlp_b"], np.float32),
    )
    dec_w = np.asarray(inputs["dec_w"], np.float32)
    dec_b = np.asarray(inputs["dec_b"], np.float32)

    if "fn" not in _jit_cache:
        _jit_cache["fn"] = _make_fn()
    fn = _jit_cache["fn"]

    xs = x.reshape(N_CORES, BS, CIN, H, W)
    out = fn(xs, K0, K, biases, dec_w, dec_b)
    return np.asarray(out).reshape(B, COUT, H, W).astype(np.float32)


if __name__ == "__main__":
    # quick self-check of the DFT matrices against numpy's fft
    rng = np.random.default_rng(0)
    a = rng.standard_normal((2, 3, H, W)).astype(np.float32)
    Fh_re, Fh_im, Fw_re, Fw_im, Eh_re, Eh_im, Cw_re, Cw_im = _build_dft_mats()
    t = np.einsum("bchw,hy->bcyw", a, Fh_re + 1j * Fh_im)
    xf = np.einsum("bcyw,wx->bcyx", t, Fw_re + 1j * Fw_im)
    ref = np.fft.rfft2(a)
    print("fwd rel err:", np.abs(xf - ref).max() / np.abs(ref).max())
    tt = np.einsum("boyx,yh->bohx", xf, Eh_re + 1j * Eh_im)
    back = np.einsum("bohx,xw->bohw", tt.real, Cw_re) - np.einsum(
        "bohx,xw->bohw", tt.imag, Cw_im)
    print("roundtrip rel err:", np.abs(back - a).max() / np.abs(a).max())
